# revision 1
# baseline (speedup 1.0000x reference)
"""Trainium2 Bass kernel for nn_Attention_46995532153449.

Module: qkv = x @ w_qkv; per-head scores = q k^T * hd^-0.5; softmax over the
HEAD axis (axis=1); attn = probs @ v; out = attn @ w_proj + b_proj.

Shapes: B=2, T=2048, D=1024, H=16, HD=64.

Sharding: data-parallel over (batch, query-block). Core c handles batch
c // 4 and queries [(c % 4) * 512, (c % 4 + 1) * 512). The head-axis softmax
is local because every core holds all 16 heads for its query slice. Each
core recomputes K/V for its whole batch (replicated across the 4 cores of a
batch) so no collectives are needed.

Layout choices (all picked so that no on-chip transpose is ever required,
and so that every matmul is a full-128-partition matmul — operands at
base_partition 64 fail on this hardware):
  - host feeds x^T (fp16), so QKV projections produce q^T/k^T [e, t] with
    e on partitions (lhsT = W as-is, rhs = x^T) and v [t, e] (lhsT = x^T
    tile, rhs = Wv).
  - scores^T[k, q] per head via a zero-padded q^T (qpad): for head pair pr,
    columns [0:QH] hold head 2pr's q^T at partitions 0:64 (zeros at
    64:128) and columns [QH:2QH] hold head 2pr+1's at partitions 64:128.
    One K=128 matmul per pair (lhsT = k^T pair chunk, rhs = qpad) yields
    both heads' scores^T side by side. ScalarE evacuates the scores PSUM
    with a fused scale+exp into fp16 E tiles.
  - head-axis softmax: S = sum of the 16 E tiles (VectorE log-tree),
    R = 1/S (VectorE reciprocal), P = E * R broadcast — split across
    VectorE (heads 0:8) and GpSimd (heads 8:16) to balance engine load.
  - attn^T[d, q] = v^T P^T per head: lhsT = v tile [k, 64], rhs = P^T
    [k, q]; odd heads write output partitions 64:128 (col-tiled matmuls,
    concurrent with the even head's). Per-head PSUM accumulation groups
    share a bank partition-split (verified on HW: has_written clearing is
    per partition; the simulator's bank-granular group check is skipped
    via skip_group_check). Accumulated over KB=4 key-chunk blocks in
    PSUM, then spill-added into an SBUF fp32 accumulator on VectorE.
  - out[q, e]: lhsT = attn^T tile [d, q], rhs = w_proj [d, e]. Output is in
    natural [q, e] order for a contiguous DMA; bias added during PSUM
    evacuation.

Measured on the 8-core axon trn2 target: max rel err 6.7e-4 vs a float64
reference; cost-model timeline estimate ~394 us/core.
"""

import numpy as np

import concourse.bacc as bacc
import concourse.mybir as mybir
import concourse.tile as tile
from concourse import bass_utils

B, T, D, H = 2, 2048, 1024, 16
HD = D // H          # 64
SCALE = HD ** -0.5   # 0.125
NCORES = 8
QS = B * T // NCORES  # 512 queries per core
DC = D // 128         # 8 d/e chunks of 128
TC = T // 128         # 16 key chunks of 128
QH = QS // 2          # 256, query half (PSUM budget)
KB = 4                # key chunks per attention block
NBLK = TC // KB

F16 = mybir.dt.float16
F32 = mybir.dt.float32
ADD = mybir.AluOpType.add
MULT = mybir.AluOpType.mult
EXP = mybir.ActivationFunctionType.Exp

_CACHED_NC = None


def _build_nc():
    nc = bacc.Bacc(
        "TRN2", target_bir_lowering=False, debug=False, enable_asserts=False
    )

    xT_d = nc.dram_tensor("xt", [D, T], F16, kind="ExternalInput").ap()
    xTq_d = nc.dram_tensor("xtq", [D, QS], F16, kind="ExternalInput").ap()
    wq_d = nc.dram_tensor("wq", [D, D], F16, kind="ExternalInput").ap()
    wk_d = nc.dram_tensor("wk", [D, D], F16, kind="ExternalInput").ap()
    wv_d = nc.dram_tensor("wv", [D, D], F16, kind="ExternalInput").ap()
    wp_d = nc.dram_tensor("wp", [D, D], F16, kind="ExternalInput").ap()
    bias_d = nc.dram_tensor("bias", [128, D], F32, kind="ExternalInput").ap()
    out_d = nc.dram_tensor("out", [QS, D], F32, kind="ExternalOutput").ap()

    def chunked(ap):  # [(c p), f] -> [p, c, f]
        return ap.rearrange("(c p) f -> p c f", p=128)

    with tile.TileContext(nc) as tc:
        with tc.tile_pool(name="persist", bufs=1) as pp:
            kT = pp.tile([128, DC, T], F16)      # k^T: [e, t], e-chunk major
            v_sb = pp.tile([128, TC, D], F16)    # v: [t, e], t-chunk major
            # zero-padded q^T: for head pair pr and query half sel, columns
            # [0:QH] hold head 2pr's q^T at partitions 0:64 (zeros below),
            # columns [QH:2QH] hold head 2pr+1's at partitions 64:128.
            # Keeps every scores matmul a full-128-partition K=128 matmul
            # (operands at base_partition 64 fail on hardware).
            qpad = pp.tile([128, DC, 2, 2 * QH], F16)
            aT = pp.tile([128, DC, QS], F16)     # attn^T: [d, q]
            wp_sb = pp.tile([128, DC, D], F16)
            bi_sb = pp.tile([128, D], F32)

            nc.gpsimd.memset(qpad, 0.0)
            nc.sync.dma_start(wp_sb, chunked(wp_d))
            nc.sync.dma_start(bi_sb, bias_d)

            # ---------------- Phase 1: QKV projections ----------------
            with tc.tile_pool(name="ph1x", bufs=1) as p1x:
                xT = p1x.tile([128, DC, T], F16)

                with (
                    tc.tile_pool(name="ph1q", bufs=1) as p1q,
                    tc.tile_pool(name="ppsq", bufs=4, space="PSUM") as ppsq,
                ):
                    xTq = p1q.tile([128, DC, QS], F16)
                    wq_sb = p1q.tile([128, DC, D], F16)
                    # Q's inputs first: the DMA ring is FIFO and these gate
                    # the kernel's first matmuls; the big x^T transfer follows
                    nc.sync.dma_start(xTq, chunked(xTq_d))
                    nc.sync.dma_start(wq_sb, chunked(wq_d))
                    nc.sync.dma_start(xT, chunked(xT_d))
                    # q^T[e, q] for this core's q-slice, written into the
                    # zero-padded layout (4 partition/half-sliced copies)
                    for ej in range(DC):
                        ps = ppsq.tile([128, 512], F32, tag="ps")
                        for jd in range(DC):
                            nc.tensor.matmul(
                                ps,
                                lhsT=wq_sb[:, jd, ej * 128:(ej + 1) * 128],
                                rhs=xTq[:, jd, :],
                                start=(jd == 0),
                                stop=(jd == DC - 1),
                            )
                        for sel in range(2):
                            nc.scalar.copy(
                                qpad[0:64, ej, sel, 0:QH],
                                ps[0:64, sel * QH:(sel + 1) * QH],
                            )
                            nc.scalar.copy(
                                qpad[64:128, ej, sel, QH:2 * QH],
                                ps[64:128, sel * QH:(sel + 1) * QH],
                            )

                with (
                    tc.tile_pool(name="ph1k", bufs=1) as p1k,
                    tc.tile_pool(name="ppsk", bufs=4, space="PSUM") as ppsk,
                ):
                    wk_sb = p1k.tile([128, DC, D], F16)
                    nc.sync.dma_start(wk_sb, chunked(wk_d))
                    # k^T[e, t] for the whole batch (tj outer: early key
                    # chunks complete first so attention can start sooner)
                    for tj in range(T // 512):
                        for ej in range(DC):
                            ps = ppsk.tile([128, 512], F32, tag="ps")
                            for jd in range(DC):
                                nc.tensor.matmul(
                                    ps,
                                    lhsT=wk_sb[:, jd, ej * 128:(ej + 1) * 128],
                                    rhs=xT[:, jd, tj * 512:(tj + 1) * 512],
                                    start=(jd == 0),
                                    stop=(jd == DC - 1),
                                )
                            nc.scalar.copy(
                                kT[:, ej, tj * 512:(tj + 1) * 512], ps
                            )

                with (
                    tc.tile_pool(name="ph1v", bufs=1) as p1v,
                    tc.tile_pool(name="ppsv", bufs=4, space="PSUM") as ppsv,
                ):
                    wv_sb = p1v.tile([128, DC, D], F16)
                    nc.sync.dma_start(wv_sb, chunked(wv_d))
                    # v[t, e] for the whole batch
                    for tj in range(TC):
                        for eh in range(2):
                            ps = ppsv.tile([128, 512], F32, tag="ps")
                            for jd in range(DC):
                                nc.tensor.matmul(
                                    ps,
                                    lhsT=xT[:, jd, tj * 128:(tj + 1) * 128],
                                    rhs=wv_sb[:, jd, eh * 512:(eh + 1) * 512],
                                    start=(jd == 0),
                                    stop=(jd == DC - 1),
                                )
                            nc.vector.tensor_copy(
                                v_sb[:, tj, eh * 512:(eh + 1) * 512], ps
                            )

            # ---------------- Phase 2: attention ----------------
            with (
                tc.tile_pool(name="attps", bufs=2, space="PSUM") as aps,
                tc.tile_pool(name="scps", bufs=2, space="PSUM") as sps,
                tc.tile_pool(name="ework", bufs=2) as epool,
                tc.tile_pool(name="swork", bufs=2) as spool,
                tc.tile_pool(name="accp", bufs=1) as accpool,
            ):
                for qh in range(2):
                    acc = accpool.tile([128, DC, QH], F32, tag="acc")
                    for blk in range(NBLK):
                        Eb = epool.tile([128, KB, H, QH], F16, tag="Eb")
                        for kcl in range(KB):
                            kc = blk * KB + kcl
                            for g in range(4):  # 4 heads per PSUM tile
                                sc = sps.tile([128, 4 * QH], F32, tag="sc")
                                for i in range(2):  # head pairs 2g, 2g+1
                                    pr = 2 * g + i
                                    nc.tensor.matmul(
                                        sc[:, i * 2 * QH:(i + 1) * 2 * QH],
                                        lhsT=kT[:, pr,
                                                kc * 128:(kc + 1) * 128],
                                        rhs=qpad[:, pr, qh, :],
                                        start=True,
                                        stop=True,
                                    )
                                # fused PSUM evacuation + scale + exp
                                nc.scalar.activation(
                                    Eb[:, kcl, 4 * g:4 * g + 4, :],
                                    sc,
                                    EXP,
                                    scale=SCALE,
                                )
                            # S = sum over heads (log tree), R = 1/S, P = E*R
                            E = Eb[:, kcl]
                            tmp = spool.tile([128, H // 2, QH], F16, tag="tmp")
                            nc.vector.tensor_tensor(
                                tmp, E[:, 0:8], E[:, 8:16], ADD
                            )
                            nc.vector.tensor_tensor(
                                tmp[:, 0:4], tmp[:, 0:4], tmp[:, 4:8], ADD
                            )
                            nc.vector.tensor_tensor(
                                tmp[:, 0:2], tmp[:, 0:2], tmp[:, 2:4], ADD
                            )
                            nc.vector.tensor_tensor(
                                tmp[:, 0:1], tmp[:, 0:1], tmp[:, 1:2], ADD
                            )
                            r = spool.tile([128, 1, QH], F16, tag="r")
                            with nc.allow_low_precision(
                                reason="softmax denominator reciprocal in fp16"
                            ):
                                nc.vector.reciprocal(r, tmp[:, 0:1])
                            nc.vector.tensor_tensor(
                                E[:, 0:8], E[:, 0:8],
                                r.to_broadcast([128, 8, QH]), MULT
                            )
                            nc.gpsimd.tensor_tensor(
                                E[:, 8:16], E[:, 8:16],
                                r.to_broadcast([128, 8, QH]), MULT
                            )
                        # attn^T: 4 waves x 2 d-chunks; one accumulation
                        # group per full PSUM bank (128 partitions), two
                        # zero-padded per-head matmuls per key chunk. 2-bank
                        # wave tiles with bufs=2 so the next wave's matmuls
                        # overlap this wave's VectorE spill-add.
                        for w in range(4):
                            ps = aps.tile([128, 2, 2 * QH], F32, tag="wv")
                            for kcl in range(KB):
                                kc = blk * KB + kcl
                                for jdl in range(2):
                                    for par in range(2):
                                        h = 4 * w + 2 * jdl + par
                                        lo = par * 64
                                        nc.tensor.matmul(
                                            ps[lo:lo + 64, jdl, 0:QH],
                                            lhsT=v_sb[:, kc,
                                                      h * 64:(h + 1) * 64],
                                            rhs=Eb[:, kcl, h, :],
                                            start=(kcl == 0),
                                            stop=(kcl == KB - 1),
                                            skip_group_check=True,
                                        )
                            if blk == 0:
                                nc.vector.tensor_copy(
                                    acc[:, 2 * w:2 * w + 2, :], ps[:, :, 0:QH]
                                )
                            elif blk == NBLK - 1:
                                # final spill writes the fp16 attn^T tile
                                # directly (saves a ScalarE conversion pass)
                                nc.vector.tensor_tensor(
                                    aT[:, 2 * w:2 * w + 2,
                                       qh * QH:(qh + 1) * QH],
                                    ps[:, :, 0:QH],
                                    acc[:, 2 * w:2 * w + 2, :],
                                    ADD,
                                )
                            else:
                                nc.vector.tensor_tensor(
                                    acc[:, 2 * w:2 * w + 2, :],
                                    ps[:, :, 0:QH],
                                    acc[:, 2 * w:2 * w + 2, :],
                                    ADD,
                                )


            # ---------------- Phase 3: output projection ----------------
            out_ch = chunked(out_d)  # [128, QS//128, D]
            with (
                tc.tile_pool(name="prj", bufs=2, space="PSUM") as prj,
                tc.tile_pool(name="outp", bufs=2) as opool,
            ):
                for qs in range(QS // 128):
                    for eh in range(2):
                        pm = prj.tile([128, 512], F32, tag="pm")
                        for jd in range(DC):
                            nc.tensor.matmul(
                                pm,
                                lhsT=aT[:, jd, qs * 128:(qs + 1) * 128],
                                rhs=wp_sb[:, jd, eh * 512:(eh + 1) * 512],
                                start=(jd == 0),
                                stop=(jd == DC - 1),
                            )
                        ot = opool.tile([128, 512], F32, tag="ot")
                        nc.vector.tensor_tensor(
                            ot, pm, bi_sb[:, eh * 512:(eh + 1) * 512], ADD
                        )
                        nc.sync.dma_start(
                            out_ch[:, qs, eh * 512:(eh + 1) * 512], ot
                        )

    nc.compile()
    return nc


def get_nc():
    global _CACHED_NC
    if _CACHED_NC is None:
        _CACHED_NC = _build_nc()
    return _CACHED_NC


def kernel(x, w_qkv, w_proj, b_proj, _trace=False, _tmpdir=None):
    x = np.asarray(x, dtype=np.float32)
    w_qkv = np.asarray(w_qkv, dtype=np.float32)
    w_proj = np.asarray(w_proj, dtype=np.float32)
    b_proj = np.asarray(b_proj, dtype=np.float32)

    # Host-side layout prep: transpose + fp16 casts + shard.
    xT = [np.ascontiguousarray(x[b].T).astype(np.float16) for b in range(B)]
    wq = np.ascontiguousarray(w_qkv[:, 0:D]).astype(np.float16)
    wk = np.ascontiguousarray(w_qkv[:, D:2 * D]).astype(np.float16)
    wv = np.ascontiguousarray(w_qkv[:, 2 * D:3 * D]).astype(np.float16)
    wp = w_proj.astype(np.float16)
    bias = np.ascontiguousarray(
        np.broadcast_to(b_proj, (128, D))
    ).astype(np.float32)

    in_maps = []
    for c in range(NCORES):
        b = c // (NCORES // B)
        qofs = (c % (NCORES // B)) * QS
        in_maps.append(
            {
                "xt": xT[b],
                "xtq": np.ascontiguousarray(xT[b][:, qofs:qofs + QS]),
                "wq": wq,
                "wk": wk,
                "wv": wv,
                "wp": wp,
                "bias": bias,
            }
        )

    nc = get_nc()
    res = bass_utils.run_bass_kernel_spmd(
        nc,
        in_maps,
        core_ids=list(range(NCORES)),
        trace=_trace,
        tmpdir=_tmpdir,
    )

    out = np.empty((B, T, D), dtype=np.float32)
    for c in range(NCORES):
        b = c // (NCORES // B)
        qofs = (c % (NCORES // B)) * QS
        out[b, qofs:qofs + QS] = res.results[c]["out"]
    if _trace:
        kernel._last_results = res
    return out



# revision 7
# speedup vs baseline: 1.0893x; 1.0893x over previous
"""Trainium2 Bass kernel for nn_Attention_46995532153449 (v2).

Module: qkv = x @ w_qkv; per-head scores = q k^T * hd^-0.5; softmax over the
HEAD axis (axis=1); attn = probs @ v; out = attn @ w_proj + b_proj.

Shapes: B=2, T=2048, D=1024, H=16, HD=64.

Sharding: data-parallel over (batch, query-block). Core c handles batch
c // 4 and queries [(c % 4) * 512, (c % 4 + 1) * 512). The head-axis softmax
is local (every core holds all 16 heads for its query slice); each core
recomputes K/V for its whole batch, so no collectives.

v2 design (vs the v1 baseline):
  - K/V projections run as hi/lo-split fp8e4m3 DoubleRow matmuls: with
    A=fp8(4w), B=fp8(x/4), the three accumulated products
    A^T B + fp8(A/16)^T fp8(-16 eps_x) + fp8(-16 eps_w)^T fp8(B/16)
    reproduce w^T x to ~0.2-0.3% while running ~8/3x cheaper than fp16 on
    the tensor engine (DoubleRow contracts 2x128 per instruction at 0.5
    cycles/row). fp8 operands are host-prepared with scales chosen to
    keep every tensor in e4m3's normal range.
  - Attention runs in 4 query-passes of 128 queries; pass 0 is
    interleaved with the K/V production so the tensor engine never
    starves. Per (key chunk, pass): 8 zero-padded head-pair score
    matmuls (fp16) fill one 4-bank PSUM tile; a single [128, 2048]
    ScalarE exp evacuates it; VectorE tree-sums the 16 heads and takes
    the reciprocal; the normalize multiply E *= R splits VectorE/GpSimd.
  - attn accumulates as [q, d] (lhsT = P^T chunk, rhs = v chunk, N=64)
    over ALL 16 key chunks in one 2-bank PSUM tile (half the
    tensor-engine cost of the v1 [d, q] layout, no spill-adds), then is
    PE-transposed to [d, q] for the output projection.
"""

import numpy as np
from ml_dtypes import float8_e4m3fn

import concourse.bacc as bacc
import concourse.mybir as mybir
import concourse.tile as tile
from concourse import bass_utils
from concourse.masks import make_identity

B, T, D, H = 2, 2048, 1024, 16
HD = D // H          # 64
SCALE = HD ** -0.5   # 0.125
NCORES = 8
QS = B * T // NCORES  # 512 queries per core
DC = D // 128         # 8 d/e chunks of 128
TC = T // 128         # 16 key chunks of 128
QP = 4                # query passes of 128
XS = 256              # x streaming slice width (t)
NXS = T // XS         # 8 slices
LAG = 3               # attn matmuls trail scores by LAG key chunks

F8 = mybir.dt.float8e4
F16 = mybir.dt.float16
F32 = mybir.dt.float32
ADD = mybir.AluOpType.add
MULT = mybir.AluOpType.mult
EXP = mybir.ActivationFunctionType.Exp
DR = mybir.MatmulPerfMode.DoubleRow

POOL_PR0 = 0   # pr groups (of 8) whose normalize runs on GpSimd in pass 0
POOL_PR = 0    # same for passes 1..3

_CACHED_NC = None


def _build_nc():
    nc = bacc.Bacc(
        "TRN2", target_bir_lowering=False, debug=False, enable_asserts=False
    )

    # fp8 operand variants for the hi/lo DoubleRow projections (host-prepped)
    x1_d = nc.dram_tensor("x1", [D, T], F8, kind="ExternalInput").ap()
    x2_d = nc.dram_tensor("x2", [D, T], F8, kind="ExternalInput").ap()
    x3_d = nc.dram_tensor("x3", [D, T], F8, kind="ExternalInput").ap()
    wk_d = [nc.dram_tensor(f"wk{i}", [D, D], F8, kind="ExternalInput").ap()
            for i in range(3)]
    wv_d = [nc.dram_tensor(f"wv{i}", [D, D], F8, kind="ExternalInput").ap()
            for i in range(3)]
    xtq_d = nc.dram_tensor("xtq", [D, QS], F16, kind="ExternalInput").ap()
    wq_d = nc.dram_tensor("wq", [D, D], F16, kind="ExternalInput").ap()
    wp_d = nc.dram_tensor("wp", [D, D], F16, kind="ExternalInput").ap()
    bias_d = nc.dram_tensor("bias", [128, D], F32, kind="ExternalInput").ap()
    out_d = nc.dram_tensor("out", [QS, D], F32, kind="ExternalOutput").ap()

    def chunked(ap):  # [(c p), f] -> [p, c, f]
        return ap.rearrange("(c p) f -> p c f", p=128)

    with tile.TileContext(nc) as tc:
        with tc.tile_pool(name="persist", bufs=1) as pp, \
             tc.tile_pool(name="xs", bufs=2) as xpool, \
             tc.tile_pool(name="scp", bufs=1, space="PSUM") as scps, \
             tc.tile_pool(name="accp", bufs=1, space="PSUM") as accps, \
             tc.tile_pool(name="utp", bufs=2, space="PSUM") as utps:
            kT = pp.tile([128, DC, T], F16)      # k^T: [e, t], e-chunk major
            v_sb = pp.tile([128, TC, D], F16)    # v: [t, e], t-chunk major
            # zero-padded q^T: per (pr, qp): col block 0 holds head 2pr's
            # q^T on partitions 0:64 (zeros below), block 1 holds head
            # 2pr+1's on partitions 64:128 (zeros above).
            qpad = pp.tile([128, DC, QP, 2, 128], F16)
            wk_sb = [pp.tile([128, DC, D], F8, name=f"wk{i}_sb")
                     for i in range(3)]
            wv_sb = [pp.tile([128, DC, D], F8, name=f"wv{i}_sb")
                     for i in range(3)]
            bi_sb = pp.tile([128, D], F32)
            aT = pp.tile([128, DC, QS], F16)     # attn^T: [d, q]
            ident = pp.tile([128, 128], F16)

            nc.gpsimd.memset(qpad, 0.0)
            make_identity(nc, ident)
            for i in range(3):
                nc.sync.dma_start(wk_sb[i], chunked(wk_d[i]))
            for i in range(3):
                nc.sync.dma_start(wv_sb[i], chunked(wv_d[i]))
            nc.sync.dma_start(bi_sb, bias_d)

            # x streaming slices: 3 fp8 variants per XS-wide t-slice
            def dma_xslice(sl):
                xt = xpool.tile([128, 3, DC, XS], F8, tag="xs", name="xt")
                for i, xd in enumerate((x1_d, x2_d, x3_d)):
                    nc.sync.dma_start(
                        xt[:, i],
                        chunked(xd)[:, :, sl * XS:(sl + 1) * XS],
                    )
                return xt

            # ---------------- Q projection (fp16) -> qpad ----------------
            with tc.tile_pool(name="phq", bufs=1) as pq:
                xTq = pq.tile([128, DC, QS], F16)
                wq_sb = pq.tile([128, DC, D], F16)
                nc.sync.dma_start(xTq, chunked(xtq_d))
                nc.sync.dma_start(wq_sb, chunked(wq_d))
                xsl0 = dma_xslice(0)
                for ej in range(DC):
                    ps = utps.tile([128, QS], F32, tag="ut", name="psq")
                    for jd in range(DC):
                        nc.tensor.matmul(
                            ps,
                            lhsT=wq_sb[:, jd, ej * 128:(ej + 1) * 128],
                            rhs=xTq[:, jd, :],
                            start=(jd == 0),
                            stop=(jd == DC - 1),
                        )
                    # write into the zero-padded layout (partition-preserving)
                    nc.scalar.copy(
                        qpad[0:64, ej, :, 0, :],
                        ps[0:64, :].rearrange("p (a b) -> p a b", b=128),
                    )
                    nc.scalar.copy(
                        qpad[64:128, ej, :, 1, :],
                        ps[64:128, :].rearrange("p (a b) -> p a b", b=128),
                    )

            # ---------------- work pools (attention + output) -------------
            with tc.tile_pool(name="work", bufs=1) as wk_pool, \
                 tc.tile_pool(name="ework", bufs=LAG + 1) as epool, \
                 tc.tile_pool(name="sm", bufs=3) as smpool, \
                 tc.tile_pool(name="aq", bufs=2) as aqpool, \
                 tc.tile_pool(name="ost", bufs=2) as ostpool:
                wp_sb = wk_pool.tile([128, DC, D], F16)
                nc.sync.dma_start(wp_sb, chunked(wp_d))

                def kproj(sl, xt):
                    # k^T [e, t-slice] via 3 hi/lo DoubleRow products
                    for ej in range(DC):
                        ps = utps.tile([128, XS], F32, tag="ut", name="psk")
                        first = True
                        for vi in range(3):
                            for jd in range(0, DC, 2):
                                nc.tensor.matmul(
                                    ps,
                                    lhsT=wk_sb[vi][:, jd:jd + 2,
                                                   ej * 128:(ej + 1) * 128],
                                    rhs=xt[:, vi, jd:jd + 2, :],
                                    start=first,
                                    stop=(vi == 2 and jd == DC - 2),
                                    perf_mode=DR,
                                )
                                first = False
                        nc.scalar.copy(
                            kT[:, ej, sl * XS:(sl + 1) * XS], ps
                        )

                def vproj(sl, xt):
                    # v [t-slice, e] via the same 3 products (operands swap:
                    # lhsT = x variant, rhs = w variant; the correction
                    # algebra is symmetric under transposition)
                    for tv in range(XS // 128):
                        tch = (sl * XS) // 128 + tv
                        for eh in range(2):
                            ps = utps.tile([128, 512], F32, tag="ut",
                                           name="psv")
                            first = True
                            for vi in range(3):
                                for jd in range(0, DC, 2):
                                    nc.tensor.matmul(
                                        ps,
                                        lhsT=xt[:, vi, jd:jd + 2,
                                                tv * 128:(tv + 1) * 128],
                                        rhs=wv_sb[vi][:, jd:jd + 2,
                                                      eh * 512:(eh + 1) * 512],
                                        start=first,
                                        stop=(vi == 2 and jd == DC - 2),
                                        perf_mode=DR,
                                    )
                                    first = False
                            nc.vector.tensor_copy(
                                v_sb[:, tch, eh * 512:(eh + 1) * 512], ps
                            )

                acc = [None]

                def combo_scores(kc, qp):
                    sc = scps.tile([128, DC, 256], F32, tag="sc", name="sc")
                    for pr in range(DC):
                        # start=True clears has_written for the WHOLE 2KB
                        # bank; two pr-groups share each bank, so only the
                        # first (even pr) may start — the odd pr's bytes
                        # were cleared by it and overwrite cleanly.
                        nc.tensor.matmul(
                            sc[:, pr, :],
                            lhsT=kT[:, pr, kc * 128:(kc + 1) * 128],
                            rhs=qpad[:, pr, qp],
                            start=(pr % 2 == 0),
                            stop=True,
                            skip_group_check=True,
                        )
                    E = epool.tile([128, DC, 2, 128], F16, tag="E", name="E")
                    nc.scalar.activation(E, sc, EXP, scale=SCALE)
                    # head tree-sum -> Z, then R = 1/Z
                    tmp = smpool.tile([128, 4, 2, 128], F16, tag="tmp",
                                      name="tmp")
                    nc.vector.tensor_tensor(tmp, E[:, 0:4], E[:, 4:8], ADD)
                    nc.vector.tensor_tensor(
                        tmp[:, 0:2], tmp[:, 0:2], tmp[:, 2:4], ADD
                    )
                    nc.vector.tensor_tensor(
                        tmp[:, 0:1], tmp[:, 0:1], tmp[:, 1:2], ADD
                    )
                    r = smpool.tile([128, 1, 1, 128], F16, tag="r", name="r")
                    nc.vector.tensor_tensor(
                        r, tmp[:, 0:1, 0:1, :], tmp[:, 0:1, 1:2, :], ADD
                    )
                    with nc.allow_low_precision(
                        reason="softmax denominator reciprocal in fp16"
                    ):
                        nc.vector.reciprocal(r, r)
                    # normalize: E *= R (broadcast over pr and head halves)
                    pool_pr = POOL_PR0 if qp == 0 else POOL_PR
                    dve_pr = DC - pool_pr
                    if dve_pr:
                        nc.vector.tensor_tensor(
                            E[:, 0:dve_pr], E[:, 0:dve_pr],
                            r.to_broadcast([128, dve_pr, 2, 128]),
                            MULT,
                        )
                    if pool_pr:
                        nc.gpsimd.tensor_tensor(
                            E[:, dve_pr:DC], E[:, dve_pr:DC],
                            r.to_broadcast([128, pool_pr, 2, 128]),
                            MULT,
                        )
                    return E

                def combo_attn(kc, E):
                    for pr in range(DC):
                        for i in range(2):
                            h = 2 * pr + i
                            # 8 head-groups share each acc bank; a start
                            # wipes the whole bank's has_written bits, so
                            # only the first head per bank (h=0 / h=8)
                            # starts — the rest overwrite cleared bytes at
                            # kc=0 and accumulate afterwards.
                            nc.tensor.matmul(
                                acc[0][:, h, :],
                                lhsT=E[:, pr, i, :],
                                rhs=v_sb[:, kc, h * 64:(h + 1) * 64],
                                start=(kc == 0 and h % 8 == 0),
                                stop=(kc == TC - 1),
                                skip_group_check=True,
                            )

                def run_pass(qp, interleave_kv):
                    acc[0] = accps.tile([128, H, HD], F32, tag="acc",
                                        name="acc")
                    pending = []
                    xt_cur = xsl0 if interleave_kv else None
                    for kc in range(TC):
                        if interleave_kv and kc % 2 == 0:
                            sl = kc // 2
                            xt_nxt = (dma_xslice(sl + 1)
                                      if sl + 1 < NXS else None)
                            kproj(sl, xt_cur)
                            vproj(sl, xt_cur)
                            xt_cur = xt_nxt
                        E = combo_scores(kc, qp)
                        pending.append((kc, E))
                        if len(pending) > LAG:
                            combo_attn(*pending.pop(0))
                    for item in pending:
                        combo_attn(*item)
                    # evacuate the [q, d] accumulator
                    aq = aqpool.tile([128, H * HD], F16, tag="aq", name="aq")
                    nc.vector.tensor_copy(aq, acc[0])
                    return aq

                def transpose_pass(qp, aq):
                    # aq [128 q, 1024 d] -> aT [d, q-block qp]
                    for jd in range(DC):
                        pst = utps.tile([128, 128], F16, tag="ut", name="pst")
                        nc.tensor.transpose(
                            pst, aq[:, jd * 128:(jd + 1) * 128], ident
                        )
                        nc.vector.tensor_copy(
                            aT[:, jd, qp * 128:(qp + 1) * 128], pst
                        )

                aqs = []
                for qp in range(QP):
                    aq = run_pass(qp, interleave_kv=(qp == 0))
                    aqs.append((qp, aq))
                    if len(aqs) > 1:
                        transpose_pass(*aqs.pop(0))
                transpose_pass(*aqs.pop(0))

                # ---------------- output projection ----------------
                out_ch = chunked(out_d)  # [128, QS//128, D]
                for qs in range(QS // 128):
                    for eh in range(2):
                        pm = utps.tile([128, 512], F32, tag="ut", name="pm")
                        for jd in range(DC):
                            nc.tensor.matmul(
                                pm,
                                lhsT=aT[:, jd, qs * 128:(qs + 1) * 128],
                                rhs=wp_sb[:, jd, eh * 512:(eh + 1) * 512],
                                start=(jd == 0),
                                stop=(jd == DC - 1),
                            )
                        ot = ostpool.tile([128, 512], F32, tag="ot", name="ot")
                        nc.vector.tensor_tensor(
                            ot, pm, bi_sb[:, eh * 512:(eh + 1) * 512], ADD
                        )
                        nc.sync.dma_start(
                            out_ch[:, qs, eh * 512:(eh + 1) * 512], ot
                        )

    nc.compile()
    return nc


def get_nc():
    global _CACHED_NC
    if _CACHED_NC is None:
        _CACHED_NC = _build_nc()
    return _CACHED_NC


def _f8(a):
    return a.astype(float8_e4m3fn)


def _prep_hilo(w):
    """w [D, D] fp32 -> (W1, W2, W3) fp8 hi/lo variants."""
    w1 = _f8(4.0 * w)
    w1f = w1.astype(np.float32)
    w2 = _f8(w1f / 16.0)
    w3 = _f8(16.0 * (4.0 * w - w1f))
    return w1, w2, w3


def kernel(x, w_qkv, w_proj, b_proj, _trace=False, _tmpdir=None):
    x = np.asarray(x, dtype=np.float32)
    w_qkv = np.asarray(w_qkv, dtype=np.float32)
    w_proj = np.asarray(w_proj, dtype=np.float32)
    b_proj = np.asarray(b_proj, dtype=np.float32)

    # Host-side layout prep.
    xT = [np.ascontiguousarray(x[b].T) for b in range(B)]  # [D, T] fp32
    x1 = []
    x2 = []
    x3 = []
    for b in range(B):
        b1 = _f8(xT[b] / 4.0)
        b1f = b1.astype(np.float32)
        x1.append(b1)
        x2.append(_f8(16.0 * (xT[b] / 4.0 - b1f)))
        x3.append(_f8(b1f / 16.0))
    wq = np.ascontiguousarray(w_qkv[:, 0:D]).astype(np.float16)
    wk3 = _prep_hilo(np.ascontiguousarray(w_qkv[:, D:2 * D]))
    wv3 = _prep_hilo(np.ascontiguousarray(w_qkv[:, 2 * D:3 * D]))
    wp = w_proj.astype(np.float16)
    bias = np.ascontiguousarray(
        np.broadcast_to(b_proj, (128, D))
    ).astype(np.float32)

    in_maps = []
    for c in range(NCORES):
        b = c // (NCORES // B)
        qofs = (c % (NCORES // B)) * QS
        im = {
            "x1": x1[b],
            "x2": x2[b],
            "x3": x3[b],
            "xtq": np.ascontiguousarray(
                xT[b][:, qofs:qofs + QS]
            ).astype(np.float16),
            "wq": wq,
            "wp": wp,
            "bias": bias,
        }
        for i in range(3):
            im[f"wk{i}"] = wk3[i]
            im[f"wv{i}"] = wv3[i]
        in_maps.append(im)

    nc = get_nc()
    res = bass_utils.run_bass_kernel_spmd(
        nc,
        in_maps,
        core_ids=list(range(NCORES)),
        trace=_trace,
        tmpdir=_tmpdir,
    )

    out = np.empty((B, T, D), dtype=np.float32)
    for c in range(NCORES):
        b = c // (NCORES // B)
        qofs = (c % (NCORES // B)) * QS
        out[b, qofs:qofs + QS] = res.results[c]["out"]
    if _trace:
        kernel._last_results = res
    return out
